# revision 46
# baseline (speedup 1.0000x reference)
"""Trainium2 Bass kernel: GQA sliding-window attention with RoPE + attention sinks.

Problem: H=32 query heads, HKV=8 kv heads, D=128, S=2048, window=1024.
Sharding: 8 cores x (4 query heads + 1 kv head); each core runs full-sequence
banded attention for its head group; no cross-core communication.

Per-core algorithm (matmuls fp16, fp32 PSUM):
  - RoPE on DVE in natural [s, d] layout; cos table stored half-width and
    read via a stride-0 broadcast AP (rope halves share cos).
  - k and q-head-0 transposed to [d, s] via PE transposes (low latency,
    feeds the pipeline front); q heads 1-3 roped into a staging tile and
    transposed by the XBAR DMA-transpose engine (idle DMA hardware).
  - Scores TRANSPOSED: psum[kj, qi] = kT.T @ qT so exp(P^T) feeds PV
    directly as the stationary operand.
  - Causal/window masks: 0/1 multiplies on DVE over exp(P^T).
  - No max-subtraction: logits ~ N(0,1); 1/sqrt(D) folded into exp scale.
  - Softmax denominators via a ones-column in V; attention sinks added to
    the denominator by a K=1 matmul inside each PV chain.
  - PE p-state warm-up matmuls at the start; output staged in 512-row
    quads to quarter the output-DMA count.
"""

import numpy as np

H, HKV, D, S = 32, 8, 128, 2048
NCORES = 8
HPC = H // NCORES          # query heads per core (4)
WINDOW = 1024
WTILES = WINDOW // 128     # 8
NT = S // 128              # 16 s-tiles
SM_SCALE = float(1.0 / np.sqrt(D))
LAG_G = 8                  # group-steps between PV-ready and PV-emit
DRAIN_MAX = 3              # PV drains per group step
NWARM = 4

_CACHE = {}


def _region_width(t):
    return 128 * (min(t + WTILES, NT - 1) - t + 1)


def _build(repeat=1):
    import contextlib
    import concourse.mybir as mybir
    import concourse.tile as tile
    from concourse import bacc

    f32 = mybir.dt.float32
    f16 = mybir.dt.float16
    mult = mybir.AluOpType.mult
    add = mybir.AluOpType.add
    EXP = mybir.ActivationFunctionType.Exp

    nc = bacc.Bacc("TRN2", target_bir_lowering=False, debug=False,
                   num_devices=NCORES)

    q_ext = nc.declare_dram_parameter("q", [S, HPC * D], f32, isOutput=False)
    k_ext = nc.declare_dram_parameter("k", [S, D], f32, isOutput=False)
    v_ext = nc.declare_dram_parameter("v", [S, D], f32, isOutput=False)
    sink_ext = nc.declare_dram_parameter("sinks", [1, HPC], f32, isOutput=False)
    out_ext = nc.declare_dram_parameter("out", [S, HPC * D], f32, isOutput=True)

    # ---- host-precomputed constants (input-independent) ----
    inv_freq = (1.0 / (10000.0 ** (np.arange(0, D, 2, dtype=np.float32) / D)))
    ang = np.arange(S, dtype=np.float32)[:, None] * inv_freq[None, :]
    cos = np.cos(ang).astype(np.float32)            # [S, 64]
    sin = np.sin(ang).astype(np.float32)
    sinm_nat = np.concatenate([-sin, sin], axis=1)  # [S, 128] sign-folded

    def to_tiles(tab):  # [S, w] -> [128, NT*w] with s-tile T at cols T*w
        w = tab.shape[1]
        return np.ascontiguousarray(
            tab.reshape(NT, 128, w).transpose(1, 0, 2).reshape(128, NT * w))

    cos_t = to_tiles(cos).astype(np.float16)        # [128, NT*64]
    sinm_t = to_tiles(sinm_nat).astype(np.float16)  # [128, NT*128]

    jj = np.arange(128)[:, None]
    ii = np.arange(128)[None, :]
    mask_diag = np.where(jj <= ii, 1.0, 0.0).astype(np.float16)
    mask_win = np.where(jj > ii, 1.0, 0.0).astype(np.float16)
    ones_blk = np.zeros((128, 128), np.float16)
    ones_blk[0, :] = 1.0

    def chunk_tbl(c):  # cos(256) + sinm(512) for s-tiles 4c..4c+3
        return np.concatenate([cos_t[:, 256 * c:256 * (c + 1)],
                               sinm_t[:, 512 * c:512 * (c + 1)]], axis=1)

    blocks = [chunk_tbl(3), np.eye(128, dtype=np.float16),
              chunk_tbl(2), chunk_tbl(1), chunk_tbl(0),
              mask_diag, mask_win, ones_blk]
    consts = np.concatenate(blocks, axis=1)
    CC = [2432, 1664, 896, 0]          # cos chunk bases (chunk c)
    CS = [2688, 1920, 1152, 256]       # sinm chunk bases
    C_ID, C_MD, C_MW, C_ONE = 768, 3200, 3328, 3456
    consts_dram = nc.inline_tensor(consts, "consts")
    warm_dram = nc.inline_tensor(np.full((128, 128), 0.01, np.float16),
                                 "warmc")

    REV_GROUPS = [[15, 14, 13, 12], [11, 10]] + [[t] for t in range(9, -1, -1)]
    FWD_GROUPS = [[t] for t in range(10)] + [[10, 11], [12, 13, 14, 15]]
    FWD_LAST = [[t] for t in range(10)] + [[10, 11], [12, 13], [14, 15]]
    HEAD_GROUPS = [REV_GROUPS, FWD_GROUPS, FWD_GROUPS, FWD_LAST]
    OFFS = []
    for hg in HEAD_GROUPS:
        om, base = {}, 0
        for g in hg:
            for t in g:
                om[t] = base
                base += _region_width(t)
        OFFS.append(om)
    TOTW = sum(_region_width(t) for t in range(NT))
    VW = D + 1  # v tile width with ones column

    with tile.TileContext(nc) as tc:
        cst = nc.alloc_sbuf_tensor("cst", [128, consts.shape[1]], f16)
        warmc = nc.alloc_sbuf_tensor("warm_sb", [128, 128], f16)
        kT_db = [nc.alloc_sbuf_tensor(f"kT_sb{p}", [128, S], f16)
                 for p in range(2)]
        qT_db = [[nc.alloc_sbuf_tensor(f"qT{h}_sb{p}", [128, S], f16)
                  for h in range(HPC)] for p in range(2)]
        v1_db = [nc.alloc_sbuf_tensor(f"v1_sb{p}", [128, NT * VW], f16)
                 for p in range(2)]
        sink_raw_db = [nc.alloc_sbuf_tensor(f"sink_raw{p}", [1, HPC], f32)
                       for p in range(2)]
        sink_exp_db = [nc.alloc_sbuf_tensor(f"sink_exp{p}", [1, HPC], f16)
                       for p in range(2)]

        with contextlib.ExitStack() as stk:
            qk_psum = stk.enter_context(
                tc.tile_pool(name="qk_psum", bufs=2, space="PSUM"))
            sp_psum = stk.enter_context(
                tc.tile_pool(name="sp_psum", bufs=2, space="PSUM"))
            bigraw_pool = stk.enter_context(tc.tile_pool(name="bigraw", bufs=4))
            qf_pool = stk.enter_context(tc.tile_pool(name="qf16", bufs=2))
            rope_pool = stk.enter_context(tc.tile_pool(name="rope", bufs=3))
            norm_pool = stk.enter_context(tc.tile_pool(name="norm", bufs=3))
            ostage_pool = stk.enter_context(tc.tile_pool(name="ostage", bufs=5))
            expp_pool = stk.enter_context(tc.tile_pool(name="expp", bufs=2))

            # rope math: dst3 = src3*cos + rot_half(src3)*sinm, all fp16 DVE
            def rope_ops(src3, c, dst3):
                cosb = cst[:, CC[c]:CC[c] + 256] \
                    .rearrange("p (T d) -> p T d", d=64) \
                    .unsqueeze(2).broadcast_to([128, 4, 2, 64])
                sin4 = cst[:, CS[c]:CS[c] + 512] \
                    .rearrange("p (T x d) -> p T x d", x=2, d=64)
                src4 = src3.rearrange("p T (x d) -> p T x d", d=64)
                tmp1 = rope_pool.tile([128, 512], f16, tag="tmp1", name="tmp1")
                tmp2 = rope_pool.tile([128, 512], f16, tag="tmp2", name="tmp2")
                t14 = tmp1[:].rearrange("p (T x d) -> p T x d", x=2, d=64)
                t24 = tmp2[:].rearrange("p (T x d) -> p T x d", x=2, d=64)
                nc.vector.tensor_tensor(t14, src4, cosb, mult)
                nc.vector.tensor_tensor(t24, src4[:, :, ::-1, :], sin4, mult)
                dst4 = dst3.rearrange("p T (x d) -> p T x d", d=64)
                nc.vector.tensor_tensor(dst4, t14, t24, add)

            # rope + PE-transpose into dst_sb[:, dst_off:+512] (k / q head 0)
            def rope_chunk(src3, c, dst_sb, dst_off, evac_act=False):
                roped = rope_pool.tile([128, 512], f16, tag="roped",
                                       name="roped")
                rope_ops(src3, c, roped[:].rearrange("p (T d) -> p T d", d=D))
                ps = sp_psum.tile([128, 512], f16, tag="sp", name="tp")
                for a in range(4):
                    nc.tensor.transpose(ps[:, 128 * a:128 * (a + 1)],
                                        roped[:, 128 * a:128 * (a + 1)],
                                        cst[:, C_ID:C_ID + 128])
                if evac_act:
                    nc.scalar.copy(dst_sb[:, dst_off:dst_off + 512], ps[:])
                else:
                    nc.vector.tensor_copy(dst_sb[:, dst_off:dst_off + 512],
                                          ps[:])

            for _rep in range(repeat):
                kT_sb = kT_db[_rep % 2]
                qT = qT_db[_rep % 2]
                v1_sb = v1_db[_rep % 2]
                sink_raw = sink_raw_db[_rep % 2]
                sink_exp = sink_exp_db[_rep % 2]
                # input streams; DMA order tuned for time-to-first-exp
                kraw = bigraw_pool.tile([128, S], f32, tag="braw", name="kraw")
                q0raw = bigraw_pool.tile([128, S], f32, tag="braw", name="q0raw")
                TB = [(0, 896), (896, 768), (1664, 768), (2432, 768),
                      (3200, 384)]

                def tbl_dma(i):
                    a, n = TB[i]
                    nc.sync.dma_start(out=cst[:, a:a + n],
                                      in_=consts_dram.ap()[:, a:a + n])

                def kq0_dma(r0, r1):
                    nc.sync.dma_start(
                        out=kraw[:, r0:r1].rearrange("p (T d) -> p T d", d=D),
                        in_=k_ext[r0:r1, :].rearrange("(T p) d -> p T d", p=128))
                    nc.sync.dma_start(
                        out=q0raw[:, r0:r1].rearrange("p (T d) -> p T d", d=D),
                        in_=q_ext[r0:r1, 0:D].rearrange("(T p) d -> p T d", p=128))

                if _rep == 0:
                    nc.sync.dma_start(out=warmc[:], in_=warm_dram.ap()[:])
                kq0_dma(1536, 2048)
                if _rep == 0:
                    tbl_dma(0)             # chunk-3 tables + eye
                    tbl_dma(4)             # masks (small, gates first PVs)
                kq0_dma(1024, 1536)
                if _rep == 0:
                    tbl_dma(1)             # chunk-2 tables
                nc.sync.dma_start(out=sink_raw[:], in_=sink_ext[:])
                kq0_dma(512, 1024)
                if _rep == 0:
                    tbl_dma(2)
                    tbl_dma(3)
                kq0_dma(0, 512)

                # PE p-state warm-up (reads the tiny warmc const)
                if _rep == 0:
                    warm_ps = qk_psum.tile([128, 512], f32, tag="qk",
                                           name="warm")
                    wrhs = warmc[:].unsqueeze(1).broadcast_to([128, 4, 128])
                    for _ in range(NWARM):
                        nc.tensor.matmul(warm_ps[:], lhsT=warmc[:], rhs=wrhs,
                                         start=True, stop=True)

                # sink prep: exp on ACT
                nc.scalar.activation(sink_exp[:], sink_raw[:], EXP)

                qraws = {h: bigraw_pool.tile([128, S], f32, tag="braw",
                                              name=f"q{h}raw")
                         for h in (1, 2, 3)}
                vraw = bigraw_pool.tile([128, S], f32, tag="braw", name="vraw")

                def qh_dma(h, r0, r1):
                    nc.sync.dma_start(
                        out=qraws[h][:, r0:r1]
                        .rearrange("p (T d) -> p T d", d=D),
                        in_=q_ext[r0:r1, h * D:(h + 1) * D]
                        .rearrange("(T p) d -> p T d", p=128))

                qh_dma(1, 0, 1024)
                qh_dma(1, 1024, 2048)
                nc.sync.dma_start(
                    out=vraw[:].rearrange("p (T d) -> p T d", d=D),
                    in_=v_ext[:].rearrange("(T p) d -> p T d", p=128))

                # fp16 staging (persist so head-0 ropes run first, rest later)
                kf = qf_pool.tile([128, S], f16, tag="kf", name="kf")
                qf0 = qf_pool.tile([128, S], f16, tag="qf0", name="qf0")
                qfh = {h: qf_pool.tile([128, S], f16, tag=f"qfh{h}",
                                       name=f"qfh{h}") for h in (1, 2, 3)}

                # head-0-critical path (reversed regions: chunk 3 first);
                # casts on gpsimd, evac on ACT while it idles, then DVE
                for c in (3, 2, 1, 0):
                    cc = 512 * c
                    ceng = nc.vector if c == 3 else nc.gpsimd
                    ceng.tensor_copy(kf[:, cc:cc + 512], kraw[:, cc:cc + 512])
                    ceng.tensor_copy(qf0[:, cc:cc + 512],
                                     q0raw[:, cc:cc + 512])
                    rope_chunk(kf[:, cc:cc + 512]
                               .rearrange("p (T d) -> p T d", d=D),
                               c, kT_sb, cc, evac_act=(c >= 2))
                    rope_chunk(qf0[:, cc:cc + 512]
                               .rearrange("p (T d) -> p T d", d=D),
                               c, qT[0], cc, evac_act=(c >= 2))

                v13 = v1_sb[:].rearrange("p (T w) -> p T w", w=VW)
                vr3 = vraw[:].rearrange("p (T d) -> p T d", d=D)

                def cast_qh(h, c):
                    nc.gpsimd.tensor_copy(qfh[h][:, 512 * c:512 * (c + 1)],
                                          qraws[h][:, 512 * c:512 * (c + 1)])

                # per-head casts in arrival order; v casts between h1 and h2
                for c in range(4):
                    cast_qh(1, c)
                for c in range(4):
                    nc.gpsimd.tensor_copy(v13[:, 4 * c:4 * (c + 1), 0:D],
                                          vr3[:, 4 * c:4 * (c + 1), :])
                nc.gpsimd.memset(v13[:, :, D:VW], 1.0)

                # heads 1..3: rope into contiguous stage, then XBAR
                # transpose in two pieces (tiles 0-11 once chunk 2 is roped
                # so the head's first QK groups aren't gated on chunk 3).
                # h1 upfront; h2/h3 deferred into the group loop so h0's
                # masks aren't queued behind their rope work on DVE.
                def rope_h(h, c):
                    s3 = qfh[h][:, 512 * c:512 * (c + 1)] \
                        .rearrange("p (T d) -> p T d", d=D)
                    rope_ops(s3, c, s3)

                def xbar_h(h, piece):
                    a, b = (0, 1536) if piece == 0 else (1536, 2048)
                    nc.sync.dma_start_transpose(
                        qT[h][:, a:b].rearrange("p (T d) -> p T d", d=128),
                        qfh[h][:, a:b])

                for c in range(4):
                    rope_h(1, c)
                    if c == 2:
                        xbar_h(1, 0)
                xbar_h(1, 1)
                for h in (2, 3):
                    qh_dma(h, 0, 1024)
                    qh_dma(h, 1024, 2048)
                for h in (2, 3):
                    for c in range(4):
                        cast_qh(h, c)
                thunk_sched = {
                    14: [lambda: rope_h(2, 0)],
                    15: [lambda: rope_h(2, 1)],
                    16: [lambda: rope_h(2, 2)],
                    17: [lambda: rope_h(2, 3)],
                    18: [lambda: xbar_h(2, 0)],
                    19: [lambda: xbar_h(2, 1)],
                    20: [lambda: rope_h(3, 0)],
                    21: [lambda: rope_h(3, 1)],
                    22: [lambda: rope_h(3, 2)],
                    23: [lambda: rope_h(3, 3)],
                    24: [lambda: xbar_h(3, 0)],
                    25: [lambda: xbar_h(3, 1)],
                }

                # ---- attention: flat (head, key-tile) pipeline, PV lags QK
                expPs = {}
                stages = {}
                quad_cnt = {}

                def do_qk(h, group):
                    expP = expPs[h]
                    offs = OFFS[h]
                    base = 0
                    regions = []
                    for t in group:
                        regions.append((t, base, _region_width(t)))
                        base += _region_width(t)
                    wtot = base
                    ps = qk_psum.tile([128, wtot], f32, tag="qk",
                                      name=f"qk{h}_{group[0]}")
                    bank_ops = {}
                    for t, rbase, w in regions:
                        cuts = {rbase, rbase + w}
                        for b in range(512, wtot, 512):
                            if rbase < b < rbase + w:
                                cuts.add(b)
                        cs = sorted(cuts)
                        for p0, p1 in zip(cs, cs[1:]):
                            bank_ops.setdefault(p0 // 512, []).append(
                                (t, rbase, p0, p1))
                    for b, ops in sorted(bank_ops.items()):
                        for idx, (t, rbase, p0, p1) in enumerate(ops):
                            nc.tensor.matmul(
                                ps[:, p0:p1],
                                lhsT=kT_sb[:, 128 * t:128 * (t + 1)],
                                rhs=qT[h][:, 128 * t + (p0 - rbase):
                                          128 * t + (p1 - rbase)],
                                start=(idx == 0), stop=(idx == len(ops) - 1))
                    o0 = offs[group[0]]
                    nc.scalar.activation(expP[:, o0:o0 + wtot], ps[:, 0:wtot],
                                         EXP, scale=SM_SCALE)
                    for t, rbase, w in regions:
                        o = offs[t]
                        if t + WTILES <= NT - 1:
                            blk = expP[:, o:o + w] \
                                .rearrange("p (a b) -> p a b", b=128)[:, 0:9:8, :]
                            msk = cst[:, C_MD:C_MD + 256] \
                                .rearrange("p (a b) -> p a b", b=128)
                            nc.vector.tensor_tensor(blk, blk, msk, mult)
                        else:
                            nc.vector.tensor_tensor(
                                expP[:, o:o + 128], expP[:, o:o + 128],
                                cst[:, C_MD:C_MD + 128], mult)

                def do_pv(h, qt):
                    expP = expPs[h]
                    offs = OFFS[h]
                    qd = qt // 4
                    if (h, qd) not in stages:
                        stages[(h, qd)] = ostage_pool.tile(
                            [128, 4 * D], f32, tag="ost", name=f"ost{h}_{qd}")
                    stage = stages[(h, qd)]
                    t_lo = max(0, qt - WTILES)
                    po = sp_psum.tile([128, VW], f32, tag="sp",
                                      name=f"pv{h}_{qt}")
                    single = qt == t_lo
                    for t in range(t_lo, qt + 1):
                        nc.tensor.matmul(
                            po[:],
                            lhsT=expP[:, offs[t] + 128 * (qt - t):
                                      offs[t] + 128 * (qt - t) + 128],
                            rhs=v1_sb[:, t * VW:(t + 1) * VW],
                            start=(t == t_lo),
                            stop=(t == qt) and not single)
                        if t == t_lo:
                            nc.tensor.matmul(po[:, D:D + 1],
                                             lhsT=cst[0:1, C_ONE:C_ONE + 128],
                                             rhs=sink_exp[0:1, h:h + 1],
                                             start=False, stop=single)
                    recip = norm_pool.tile([128, 1], f32, tag="recip",
                                           name="recip")
                    nc.vector.reciprocal(recip[:], po[:, D:D + 1])
                    j = qt % 4
                    nc.vector.tensor_scalar(stage[:, D * j:D * (j + 1)],
                                            po[:, 0:D], recip[:], None, mult)
                    filled = quad_cnt.setdefault((h, qd), set())
                    filled.add(j)
                    if h == HPC - 1 and qd == 3:
                        for lo, sl in ((0, (0, 1)), (256, (2, 3))):
                            key = ("sent", lo)
                            if key not in filled \
                                    and all(s in filled for s in sl):
                                filled.add(key)
                                r0 = 512 * qd + lo
                                nc.sync.dma_start(
                                    out=out_ext[r0:r0 + 256,
                                                D * h:D * (h + 1)]
                                    .rearrange("(T p) d -> p T d", p=128),
                                    in_=stage[:, D * sl[0]:D * (sl[1] + 1)]
                                    .rearrange("p (T d) -> p T d", d=D))
                    elif len(filled) == 4:
                        nc.sync.dma_start(
                            out=out_ext[512 * qd:512 * (qd + 1),
                                        D * h:D * (h + 1)]
                            .rearrange("(T p) d -> p T d", p=128),
                            in_=stage[:].rearrange("p (T d) -> p T d", d=D))

                steps = [(h, gi) for h in range(HPC)
                         for gi in range(len(HEAD_GROUPS[h]))]
                rlist = []
                rdone = 0
                queued = set()
                done_regions = {h: set() for h in range(HPC)}
                gstep = 0
                for h, gi in steps:
                    if gi == 0:
                        expPs[h] = expp_pool.tile([128, TOTW], f16,
                                                  tag="expp", name=f"expP{h}")
                    tail = len(steps) - gstep
                    lag_now = min(LAG_G, max(1, tail - 3))
                    dmax = DRAIN_MAX if tail > 8 else DRAIN_MAX + 4
                    drained = 0
                    while rdone < len(rlist) and drained < dmax \
                            and rlist[rdone][0] <= gstep - lag_now:
                        _, ph, pqt = rlist[rdone]
                        do_pv(ph, pqt)
                        rdone += 1
                        drained += 1
                    do_qk(h, HEAD_GROUPS[h][gi])
                    for th in thunk_sched.pop(gstep, ()):
                        th()
                    done_regions[h].update(HEAD_GROUPS[h][gi])
                    for qt in range(NT):
                        if (h, qt) in queued:
                            continue
                        if all(t in done_regions[h]
                               for t in range(max(0, qt - WTILES), qt + 1)):
                            rlist.append((gstep, h, qt))
                            queued.add((h, qt))
                    gstep += 1
                while rdone < len(rlist):
                    _, ph, pqt = rlist[rdone]
                    do_pv(ph, pqt)
                    rdone += 1

    nc.compile()
    return nc


def _get_nc(repeat=1):
    key = f"nc{repeat}"
    if key not in _CACHE:
        _CACHE[key] = _build(repeat)
    return _CACHE[key]


def kernel(q, k, v, attention_sinks, attention_window_size=1024):
    from concourse.bass_utils import run_bass_kernel_spmd

    assert int(attention_window_size) == WINDOW, "kernel compiled for window=1024"
    q = np.ascontiguousarray(np.asarray(q, dtype=np.float32))
    k = np.ascontiguousarray(np.asarray(k, dtype=np.float32))
    v = np.ascontiguousarray(np.asarray(v, dtype=np.float32))
    sinks = np.asarray(attention_sinks, dtype=np.float32).reshape(H)

    nc = _get_nc()
    in_maps = []
    for c in range(NCORES):
        in_maps.append({
            "q": np.ascontiguousarray(q[:, c * HPC * D:(c + 1) * HPC * D]),
            "k": np.ascontiguousarray(k[:, c * D:(c + 1) * D]),
            "v": np.ascontiguousarray(v[:, c * D:(c + 1) * D]),
            "sinks": np.ascontiguousarray(sinks[c * HPC:(c + 1) * HPC]
                                          .reshape(1, HPC)),
        })
    res = run_bass_kernel_spmd(nc, in_maps, core_ids=list(range(NCORES)))
    out = np.empty((S, H * D), dtype=np.float32)
    for c in range(NCORES):
        out[:, c * HPC * D:(c + 1) * HPC * D] = res.results[c]["out"]
    return out


# revision 50
# speedup vs baseline: 1.6561x; 1.6561x over previous
"""Trainium2 Bass kernel: GQA sliding-window attention with RoPE + attention sinks.

Problem: H=32 query heads, HKV=8 kv heads, D=128, S=2048, window=1024.
Sharding: 8 cores x (4 query heads + 1 kv head); each core runs full-sequence
banded attention for its head group; no cross-core communication.

Per-core algorithm (matmuls fp16, fp32 PSUM):
  - RoPE on DVE in natural [s, d] layout; cos table stored half-width and
    read via a stride-0 broadcast AP (rope halves share cos).
  - k and q-head-0 transposed to [d, s] via PE transposes (low latency,
    feeds the pipeline front); q heads 1-3 roped into a staging tile and
    transposed by the XBAR DMA-transpose engine (idle DMA hardware).
  - Scores TRANSPOSED: psum[kj, qi] = kT.T @ qT so exp(P^T) feeds PV
    directly as the stationary operand.
  - Causal/window masks: 0/1 multiplies on DVE over exp(P^T).
  - No max-subtraction: logits ~ N(0,1); 1/sqrt(D) folded into exp scale.
  - Softmax denominators via a ones-column in V; attention sinks added to
    the denominator by a K=1 matmul inside each PV chain.
  - PE p-state warm-up matmuls at the start; output staged in 512-row
    quads to quarter the output-DMA count.
"""

import numpy as np

H, HKV, D, S = 32, 8, 128, 2048
NCORES = 8
HPC = H // NCORES          # query heads per core (4)
WINDOW = 1024
WTILES = WINDOW // 128     # 8
NT = S // 128              # 16 s-tiles
SM_SCALE = float(1.0 / np.sqrt(D))
LAG_G = 4                  # window-steps between PV-ready and PV-emit
DRAIN_MAX = 3              # PV drains per window step
NWARM = 4

_CACHE = {}


def _region_width(t):
    return 128 * (min(t + WTILES, NT - 1) - t + 1)


def _build(repeat=1):
    import contextlib
    import concourse.mybir as mybir
    import concourse.tile as tile
    from concourse import bacc

    f32 = mybir.dt.float32
    f16 = mybir.dt.float16
    mult = mybir.AluOpType.mult
    add = mybir.AluOpType.add
    EXP = mybir.ActivationFunctionType.Exp

    nc = bacc.Bacc("TRN2", target_bir_lowering=False, debug=False,
                   num_devices=NCORES)

    q_ext = nc.declare_dram_parameter("q", [S, HPC * D], f32, isOutput=False)
    k_ext = nc.declare_dram_parameter("k", [S, D], f32, isOutput=False)
    v_ext = nc.declare_dram_parameter("v", [S, D], f32, isOutput=False)
    sink_ext = nc.declare_dram_parameter("sinks", [1, HPC], f32, isOutput=False)
    out_ext = nc.declare_dram_parameter("out", [S, HPC * D], f32, isOutput=True)

    # ---- host-precomputed constants (input-independent) ----
    inv_freq = (1.0 / (10000.0 ** (np.arange(0, D, 2, dtype=np.float32) / D)))
    ang = np.arange(S, dtype=np.float32)[:, None] * inv_freq[None, :]
    cos = np.cos(ang).astype(np.float32)            # [S, 64]
    sin = np.sin(ang).astype(np.float32)
    sinm_nat = np.concatenate([-sin, sin], axis=1)  # [S, 128] sign-folded

    def to_tiles(tab):  # [S, w] -> [128, NT*w] with s-tile T at cols T*w
        w = tab.shape[1]
        return np.ascontiguousarray(
            tab.reshape(NT, 128, w).transpose(1, 0, 2).reshape(128, NT * w))

    cos_t = to_tiles(cos).astype(np.float16)        # [128, NT*64]
    sinm_t = to_tiles(sinm_nat).astype(np.float16)  # [128, NT*128]

    jj = np.arange(128)[:, None]
    ii = np.arange(128)[None, :]
    mask_diag = np.where(jj <= ii, 1.0, 0.0).astype(np.float16)
    mask_win = np.where(jj > ii, 1.0, 0.0).astype(np.float16)
    ones_blk = np.zeros((128, 128), np.float16)
    ones_blk[0, :] = 1.0

    def chunk_tbl(c):  # cos(256) + sinm(512) for s-tiles 4c..4c+3
        return np.concatenate([cos_t[:, 256 * c:256 * (c + 1)],
                               sinm_t[:, 512 * c:512 * (c + 1)]], axis=1)

    blocks = [chunk_tbl(3), np.eye(128, dtype=np.float16),
              chunk_tbl(2), chunk_tbl(1), chunk_tbl(0),
              mask_diag, mask_win, ones_blk]
    consts = np.concatenate(blocks, axis=1)
    CC = [2432, 1664, 896, 0]          # cos chunk bases (chunk c)
    CS = [2688, 1920, 1152, 256]       # sinm chunk bases
    C_ID, C_MD, C_MW, C_ONE = 768, 3200, 3328, 3456
    consts_dram = nc.inline_tensor(consts, "consts")
    warm_dram = nc.inline_tensor(np.full((128, 128), 0.01, np.float16),
                                 "warmc")

    REV_ORDER = list(range(15, -1, -1))
    FWD_ORDER = list(range(16))
    HEAD_ORDER = [REV_ORDER, FWD_ORDER, FWD_ORDER, FWD_ORDER]
    TOTW = sum(_region_width(t) for t in range(NT))
    OFFS = []
    for order in HEAD_ORDER:
        om, base = {}, 0
        for t in order:
            om[t] = base
            base += _region_width(t)
        OFFS.append(om)
    WIN_REV = [1280, 1408] + [1536] * 7 + [384]
    WIN_FWD = [1536] * 9
    HEAD_WINS = [WIN_REV, WIN_FWD, WIN_FWD, [1536] * 8 + [768, 768]]
    VW = D + 1  # v tile width with ones column

    with tile.TileContext(nc) as tc:
        cst = nc.alloc_sbuf_tensor("cst", [128, consts.shape[1]], f16)
        warmc = nc.alloc_sbuf_tensor("warm_sb", [128, 128], f16)
        kT_db = [nc.alloc_sbuf_tensor(f"kT_sb{p}", [128, S], f16)
                 for p in range(2)]
        qT_db = [[nc.alloc_sbuf_tensor(f"qT{h}_sb{p}", [128, S], f16)
                  for h in range(HPC)] for p in range(2)]
        v1_db = [nc.alloc_sbuf_tensor(f"v1_sb{p}", [128, NT * VW], f16)
                 for p in range(2)]
        sink_raw_db = [nc.alloc_sbuf_tensor(f"sink_raw{p}", [1, HPC], f32)
                       for p in range(2)]
        sink_exp_db = [nc.alloc_sbuf_tensor(f"sink_exp{p}", [1, HPC], f16)
                       for p in range(2)]

        with contextlib.ExitStack() as stk:
            qk_psum = stk.enter_context(
                tc.tile_pool(name="qk_psum", bufs=2, space="PSUM"))
            sp_psum = stk.enter_context(
                tc.tile_pool(name="sp_psum", bufs=2, space="PSUM"))
            bigraw_pool = stk.enter_context(tc.tile_pool(name="bigraw", bufs=4))
            qf_pool = stk.enter_context(tc.tile_pool(name="qf16", bufs=2))
            rope_pool = stk.enter_context(tc.tile_pool(name="rope", bufs=3))
            norm_pool = stk.enter_context(tc.tile_pool(name="norm", bufs=3))
            ostage_pool = stk.enter_context(tc.tile_pool(name="ostage", bufs=5))
            expp_pool = stk.enter_context(tc.tile_pool(name="expp", bufs=2))

            # rope math: dst3 = src3*cos + rot_half(src3)*sinm, all fp16 DVE
            def rope_ops(src3, c, dst3):
                cosb = cst[:, CC[c]:CC[c] + 256] \
                    .rearrange("p (T d) -> p T d", d=64) \
                    .unsqueeze(2).broadcast_to([128, 4, 2, 64])
                sin4 = cst[:, CS[c]:CS[c] + 512] \
                    .rearrange("p (T x d) -> p T x d", x=2, d=64)
                src4 = src3.rearrange("p T (x d) -> p T x d", d=64)
                tmp1 = rope_pool.tile([128, 512], f16, tag="tmp1", name="tmp1")
                tmp2 = rope_pool.tile([128, 512], f16, tag="tmp2", name="tmp2")
                t14 = tmp1[:].rearrange("p (T x d) -> p T x d", x=2, d=64)
                t24 = tmp2[:].rearrange("p (T x d) -> p T x d", x=2, d=64)
                nc.vector.tensor_tensor(t14, src4, cosb, mult)
                nc.vector.tensor_tensor(t24, src4[:, :, ::-1, :], sin4, mult)
                dst4 = dst3.rearrange("p T (x d) -> p T x d", d=64)
                nc.vector.tensor_tensor(dst4, t14, t24, add)

            # rope + PE-transpose into dst_sb[:, dst_off:+512] (k / q head 0)
            def rope_chunk(src3, c, dst_sb, dst_off, evac_act=False):
                roped = rope_pool.tile([128, 512], f16, tag="roped",
                                       name="roped")
                rope_ops(src3, c, roped[:].rearrange("p (T d) -> p T d", d=D))
                ps = sp_psum.tile([128, 512], f16, tag="sp", name="tp")
                for a in range(4):
                    nc.tensor.transpose(ps[:, 128 * a:128 * (a + 1)],
                                        roped[:, 128 * a:128 * (a + 1)],
                                        cst[:, C_ID:C_ID + 128])
                if evac_act:
                    nc.scalar.copy(dst_sb[:, dst_off:dst_off + 512], ps[:])
                else:
                    nc.vector.tensor_copy(dst_sb[:, dst_off:dst_off + 512],
                                          ps[:])

            for _rep in range(repeat):
                kT_sb = kT_db[_rep % 2]
                qT = qT_db[_rep % 2]
                v1_sb = v1_db[_rep % 2]
                sink_raw = sink_raw_db[_rep % 2]
                sink_exp = sink_exp_db[_rep % 2]
                # input streams; DMA order tuned for time-to-first-exp
                kraw = bigraw_pool.tile([128, S], f32, tag="braw", name="kraw")
                q0raw = bigraw_pool.tile([128, S], f32, tag="braw", name="q0raw")
                TB = [(0, 896), (896, 768), (1664, 768), (2432, 768),
                      (3200, 384)]

                def tbl_dma(i):
                    a, n = TB[i]
                    nc.sync.dma_start(out=cst[:, a:a + n],
                                      in_=consts_dram.ap()[:, a:a + n])

                def kq0_dma(r0, r1):
                    nc.sync.dma_start(
                        out=kraw[:, r0:r1].rearrange("p (T d) -> p T d", d=D),
                        in_=k_ext[r0:r1, :].rearrange("(T p) d -> p T d", p=128))
                    nc.sync.dma_start(
                        out=q0raw[:, r0:r1].rearrange("p (T d) -> p T d", d=D),
                        in_=q_ext[r0:r1, 0:D].rearrange("(T p) d -> p T d", p=128))

                if _rep == 0:
                    nc.sync.dma_start(out=warmc[:], in_=warm_dram.ap()[:])
                kq0_dma(1536, 2048)
                if _rep == 0:
                    tbl_dma(0)             # chunk-3 tables + eye
                    tbl_dma(4)             # masks (small, gates first PVs)
                kq0_dma(1024, 1536)
                if _rep == 0:
                    tbl_dma(1)             # chunk-2 tables
                nc.sync.dma_start(out=sink_raw[:], in_=sink_ext[:])
                kq0_dma(512, 1024)
                if _rep == 0:
                    tbl_dma(2)
                    tbl_dma(3)
                kq0_dma(0, 512)

                # PE p-state warm-up (reads the tiny warmc const)
                if _rep == 0:
                    warm_ps = qk_psum.tile([128, 512], f32, tag="qk",
                                           name="warm")
                    wrhs = warmc[:].unsqueeze(1).broadcast_to([128, 4, 128])
                    for _ in range(NWARM):
                        nc.tensor.matmul(warm_ps[:], lhsT=warmc[:], rhs=wrhs,
                                         start=True, stop=True)

                # sink prep: exp on ACT
                nc.scalar.activation(sink_exp[:], sink_raw[:], EXP)

                qraws = {h: bigraw_pool.tile([128, S], f32, tag="braw",
                                              name=f"q{h}raw")
                         for h in (1, 2, 3)}
                vraw = bigraw_pool.tile([128, S], f32, tag="braw", name="vraw")

                def qh_dma(h, r0, r1):
                    nc.sync.dma_start(
                        out=qraws[h][:, r0:r1]
                        .rearrange("p (T d) -> p T d", d=D),
                        in_=q_ext[r0:r1, h * D:(h + 1) * D]
                        .rearrange("(T p) d -> p T d", p=128))

                qh_dma(1, 0, 1024)
                qh_dma(1, 1024, 2048)
                nc.sync.dma_start(
                    out=vraw[:].rearrange("p (T d) -> p T d", d=D),
                    in_=v_ext[:].rearrange("(T p) d -> p T d", p=128))

                # fp16 staging (persist so head-0 ropes run first, rest later)
                kf = qf_pool.tile([128, S], f16, tag="kf", name="kf")
                qf0 = qf_pool.tile([128, S], f16, tag="qf0", name="qf0")
                qfh = {h: qf_pool.tile([128, S], f16, tag=f"qfh{h}",
                                       name=f"qfh{h}") for h in (1, 2, 3)}

                # head-0-critical path (reversed regions: chunk 3 first);
                # casts on gpsimd, evac on ACT while it idles, then DVE
                for c in (3, 2, 1, 0):
                    cc = 512 * c
                    ceng = nc.vector if c == 3 else nc.gpsimd
                    ceng.tensor_copy(kf[:, cc:cc + 512], kraw[:, cc:cc + 512])
                    ceng.tensor_copy(qf0[:, cc:cc + 512],
                                     q0raw[:, cc:cc + 512])
                    rope_chunk(kf[:, cc:cc + 512]
                               .rearrange("p (T d) -> p T d", d=D),
                               c, kT_sb, cc, evac_act=(c >= 2))
                    rope_chunk(qf0[:, cc:cc + 512]
                               .rearrange("p (T d) -> p T d", d=D),
                               c, qT[0], cc, evac_act=(c >= 2))

                v13 = v1_sb[:].rearrange("p (T w) -> p T w", w=VW)
                vr3 = vraw[:].rearrange("p (T d) -> p T d", d=D)

                def cast_qh(h, c):
                    nc.gpsimd.tensor_copy(qfh[h][:, 512 * c:512 * (c + 1)],
                                          qraws[h][:, 512 * c:512 * (c + 1)])

                # per-head casts in arrival order; v casts between h1 and h2
                for c in range(4):
                    cast_qh(1, c)
                for c in range(4):
                    nc.gpsimd.tensor_copy(v13[:, 4 * c:4 * (c + 1), 0:D],
                                          vr3[:, 4 * c:4 * (c + 1), :])
                nc.gpsimd.memset(v13[:, :, D:VW], 1.0)

                # heads 1..3: rope into contiguous stage, then XBAR
                # transpose in two pieces (tiles 0-11 once chunk 2 is roped
                # so the head's first QK groups aren't gated on chunk 3).
                # h1 upfront; h2/h3 deferred into the group loop so h0's
                # masks aren't queued behind their rope work on DVE.
                def rope_h(h, c):
                    s3 = qfh[h][:, 512 * c:512 * (c + 1)] \
                        .rearrange("p (T d) -> p T d", d=D)
                    rope_ops(s3, c, s3)

                def xbar_h(h, piece):
                    a, b = (0, 1536) if piece == 0 else (1536, 2048)
                    nc.sync.dma_start_transpose(
                        qT[h][:, a:b].rearrange("p (T d) -> p T d", d=128),
                        qfh[h][:, a:b])

                for c in range(4):
                    rope_h(1, c)
                    if c == 2:
                        xbar_h(1, 0)
                xbar_h(1, 1)
                for h in (2, 3):
                    qh_dma(h, 0, 1024)
                    qh_dma(h, 1024, 2048)
                for h in (2, 3):
                    for c in range(4):
                        cast_qh(h, c)
                thunk_sched = {
                    11: [lambda: rope_h(2, 0)],
                    12: [lambda: rope_h(2, 1)],
                    13: [lambda: rope_h(2, 2)],
                    14: [lambda: rope_h(2, 3)],
                    15: [lambda: xbar_h(2, 0)],
                    16: [lambda: xbar_h(2, 1)],
                    20: [lambda: rope_h(3, 0)],
                    21: [lambda: rope_h(3, 1)],
                    22: [lambda: rope_h(3, 2)],
                    23: [lambda: rope_h(3, 3)],
                    24: [lambda: xbar_h(3, 0)],
                    25: [lambda: xbar_h(3, 1)],
                }

                # ---- attention: flat (head, key-tile) pipeline, PV lags QK
                expPs = {}
                stages = {}
                quad_cnt = {}

                def do_window(h, w0, w1):
                    expP = expPs[h]
                    offs = OFFS[h]
                    ps = qk_psum.tile([128, w1 - w0], f32, tag="qk",
                                      name=f"qk{h}_{w0}")
                    ended = []
                    bank_ops = {}
                    for t in HEAD_ORDER[h]:
                        o_t, wd = offs[t], _region_width(t)
                        a, b = max(o_t, w0), min(o_t + wd, w1)
                        if a >= b:
                            continue
                        if o_t + wd <= w1:
                            ended.append(t)
                        cuts = {a, b}
                        for bb in range(w0 + 512, w1, 512):
                            if a < bb < b:
                                cuts.add(bb)
                        cs = sorted(cuts)
                        for p0, p1 in zip(cs, cs[1:]):
                            bank_ops.setdefault((p0 - w0) // 512, []).append(
                                (t, o_t, p0, p1))
                    for bk, ops in sorted(bank_ops.items()):
                        for idx, (t, o_t, p0, p1) in enumerate(ops):
                            nc.tensor.matmul(
                                ps[:, p0 - w0:p1 - w0],
                                lhsT=kT_sb[:, 128 * t:128 * (t + 1)],
                                rhs=qT[h][:, 128 * t + (p0 - o_t):
                                          128 * t + (p1 - o_t)],
                                start=(idx == 0), stop=(idx == len(ops) - 1))
                    nc.scalar.activation(expP[:, w0:w1], ps[:, 0:w1 - w0],
                                         EXP, scale=SM_SCALE)
                    for t in ended:
                        o, w = offs[t], _region_width(t)
                        if t + WTILES <= NT - 1:
                            blk = expP[:, o:o + w] \
                                .rearrange("p (a b) -> p a b", b=128)[:, 0:9:8, :]
                            msk = cst[:, C_MD:C_MD + 256] \
                                .rearrange("p (a b) -> p a b", b=128)
                            nc.vector.tensor_tensor(blk, blk, msk, mult)
                        else:
                            nc.vector.tensor_tensor(
                                expP[:, o:o + 128], expP[:, o:o + 128],
                                cst[:, C_MD:C_MD + 128], mult)
                    return ended

                def do_pv(h, qt):
                    expP = expPs[h]
                    offs = OFFS[h]
                    qd = qt // 4
                    if (h, qd) not in stages:
                        stages[(h, qd)] = ostage_pool.tile(
                            [128, 4 * D], f32, tag="ost", name=f"ost{h}_{qd}")
                    stage = stages[(h, qd)]
                    t_lo = max(0, qt - WTILES)
                    po = sp_psum.tile([128, VW], f32, tag="sp",
                                      name=f"pv{h}_{qt}")
                    single = qt == t_lo
                    for t in range(t_lo, qt + 1):
                        nc.tensor.matmul(
                            po[:],
                            lhsT=expP[:, offs[t] + 128 * (qt - t):
                                      offs[t] + 128 * (qt - t) + 128],
                            rhs=v1_sb[:, t * VW:(t + 1) * VW],
                            start=(t == t_lo),
                            stop=(t == qt) and not single)
                        if t == t_lo:
                            nc.tensor.matmul(po[:, D:D + 1],
                                             lhsT=cst[0:1, C_ONE:C_ONE + 128],
                                             rhs=sink_exp[0:1, h:h + 1],
                                             start=False, stop=single)
                    recip = norm_pool.tile([128, 1], f32, tag="recip",
                                           name="recip")
                    nc.vector.reciprocal(recip[:], po[:, D:D + 1])
                    j = qt % 4
                    nc.vector.tensor_scalar(stage[:, D * j:D * (j + 1)],
                                            po[:, 0:D], recip[:], None, mult)
                    filled = quad_cnt.setdefault((h, qd), set())
                    filled.add(j)
                    if h == HPC - 1 and qd == 3:
                        for lo, sl in ((0, (0, 1)), (256, (2, 3))):
                            key = ("sent", lo)
                            if key not in filled \
                                    and all(s in filled for s in sl):
                                filled.add(key)
                                r0 = 512 * qd + lo
                                nc.sync.dma_start(
                                    out=out_ext[r0:r0 + 256,
                                                D * h:D * (h + 1)]
                                    .rearrange("(T p) d -> p T d", p=128),
                                    in_=stage[:, D * sl[0]:D * (sl[1] + 1)]
                                    .rearrange("p (T d) -> p T d", d=D))
                    elif len(filled) == 4:
                        nc.sync.dma_start(
                            out=out_ext[512 * qd:512 * (qd + 1),
                                        D * h:D * (h + 1)]
                            .rearrange("(T p) d -> p T d", p=128),
                            in_=stage[:].rearrange("p (T d) -> p T d", d=D))

                steps = []
                for h in range(HPC):
                    base = 0
                    for wd in HEAD_WINS[h]:
                        steps.append((h, base, base + wd))
                        base += wd
                rlist = []
                rdone = 0
                queued = set()
                done_regions = {h: set() for h in range(HPC)}
                gstep = 0
                for h, w0, w1 in steps:
                    if w0 == 0:
                        expPs[h] = expp_pool.tile([128, TOTW], f16,
                                                  tag="expp", name=f"expP{h}")
                    tail = len(steps) - gstep
                    lag_now = min(LAG_G, max(1, tail - 3))
                    dmax = DRAIN_MAX if tail > 8 else DRAIN_MAX + 4
                    drained = 0
                    while rdone < len(rlist) and drained < dmax \
                            and rlist[rdone][0] <= gstep - lag_now:
                        _, ph, pqt = rlist[rdone]
                        do_pv(ph, pqt)
                        rdone += 1
                        drained += 1
                    ended = do_window(h, w0, w1)
                    for th in thunk_sched.pop(gstep, ()):
                        th()
                    done_regions[h].update(ended)
                    for qt in range(NT):
                        if (h, qt) in queued:
                            continue
                        if all(t in done_regions[h]
                               for t in range(max(0, qt - WTILES), qt + 1)):
                            rlist.append((gstep, h, qt))
                            queued.add((h, qt))
                    gstep += 1
                while rdone < len(rlist):
                    _, ph, pqt = rlist[rdone]
                    do_pv(ph, pqt)
                    rdone += 1

    nc.compile()
    return nc


def _get_nc(repeat=1):
    key = f"nc{repeat}"
    if key not in _CACHE:
        _CACHE[key] = _build(repeat)
    return _CACHE[key]


def kernel(q, k, v, attention_sinks, attention_window_size=1024):
    from concourse.bass_utils import run_bass_kernel_spmd

    assert int(attention_window_size) == WINDOW, "kernel compiled for window=1024"
    q = np.ascontiguousarray(np.asarray(q, dtype=np.float32))
    k = np.ascontiguousarray(np.asarray(k, dtype=np.float32))
    v = np.ascontiguousarray(np.asarray(v, dtype=np.float32))
    sinks = np.asarray(attention_sinks, dtype=np.float32).reshape(H)

    nc = _get_nc()
    in_maps = []
    for c in range(NCORES):
        in_maps.append({
            "q": np.ascontiguousarray(q[:, c * HPC * D:(c + 1) * HPC * D]),
            "k": np.ascontiguousarray(k[:, c * D:(c + 1) * D]),
            "v": np.ascontiguousarray(v[:, c * D:(c + 1) * D]),
            "sinks": np.ascontiguousarray(sinks[c * HPC:(c + 1) * HPC]
                                          .reshape(1, HPC)),
        })
    res = run_bass_kernel_spmd(nc, in_maps, core_ids=list(range(NCORES)))
    out = np.empty((S, H * D), dtype=np.float32)
    for c in range(NCORES):
        out[:, c * HPC * D:(c + 1) * HPC * D] = res.results[c]["out"]
    return out
